# revision 22
# baseline (speedup 1.0000x reference)
"""Distributed GCN (3x GCNConv + global_max_pool + MLP head) on 8 Trainium2
NeuronCores via concourse Bass/Tile SPMD. Graph-parallel: 8 graphs/core, node
rows block-packed per core balancing per-block in-edge counts.

Per layer, edge messages are segment-summed with one-hot PE matmuls (prebuilt
S matrices shared by all three convs). Conv1 reads a host-pregathered
edge-ordered x table with plain affine DMA (the gather permutation is known
at prep time). Conv2/3 gather bf16 h' rows with SWDGE dma_gather, one
1024-row call per block (the Q7 descriptor-generation cap). Self-loops are
excluded from the gather streams: their contribution is one identity matmul
per block from an affine read of the core's own rows. bf16 h' tables are
exchanged with an 8-rank AllGather between layers; per-graph max-pool via
transpose-mode dma_gather + reduce_max; per-core MLP head on device. Host
only shards/unshards.
"""
import sys
sys.path.insert(0, "/opt/trn_rl_repo")
import numpy as np

N = 20000
E = 160000
G = 64
IN = 128
HID = 512
ACTD = 32
NC = 8
RPC = 2560           # rows per core
BPC = 20             # blocks per core
NB = NC * BPC        # 160 global blocks
NPAD = NC * RPC      # 20480


def _wrap16(a):
    """Flat index list [L] -> SWDGE idx layout [128, L/16] int16.

    dma_gather reads index i from (partition i%16, col i//16); the 16-row
    pattern is replicated across all 8 Q7 core groups (128 partitions).
    """
    a = np.asarray(a)
    assert a.size % 16 == 0
    w = a.reshape(-1, 16).T.astype(np.int16)   # [16, L/16]
    return np.tile(w, (8, 1)).copy()           # [128, L/16]


def prep(inputs):
    import ml_dtypes
    src = np.asarray(inputs["edge_index"][0], dtype=np.int64)
    dst = np.asarray(inputs["edge_index"][1], dtype=np.int64)
    batch = np.asarray(inputs["batch"], dtype=np.int64)
    tree_x = np.asarray(inputs["tree_x"], dtype=np.float32)

    indeg = np.bincount(dst, minlength=N).astype(np.float64)  # real in-edges
    deg = indeg + 1.0                                         # + self loop
    dinv = (1.0 / np.sqrt(deg)).astype(np.float32)

    gsizes = np.bincount(batch, minlength=G)
    # graphs -> cores: 8 consecutive per core; LPT fallback if any group > RPC
    groups = [list(range(8 * k, 8 * k + 8)) for k in range(NC)]
    if max(int(gsizes[g].sum()) for g in groups) > RPC:
        order = np.argsort(-gsizes)
        loads = [0] * NC
        counts = [0] * NC
        groups = [[] for _ in range(NC)]
        for g in order:
            k = min(range(NC), key=lambda i: (loads[i] if counts[i] < 8 else 1 << 60))
            groups[k].append(int(g))
            loads[k] += int(gsizes[g])
            counts[k] += 1
        assert max(loads) <= RPC, f"graph groups do not fit: {loads}"

    graph_core = np.zeros(G, dtype=np.int64)
    graph_slot = np.zeros(G, dtype=np.int64)
    for k in range(NC):
        for j, g in enumerate(groups[k]):
            graph_core[g] = k
            graph_slot[g] = j

    # --- node -> (core, block, slot) balancing per-block real in-edge load ---
    import heapq
    newid = np.full(N, -1, dtype=np.int64)
    node_core = graph_core[batch]
    max_block_load = 0
    for k in range(NC):
        nodes = np.where(node_core == k)[0]
        nodes = nodes[np.argsort(-indeg[nodes], kind="stable")]
        heap = [(0.0, 0, b) for b in range(BPC)]  # (load, count, block)
        heapq.heapify(heap)
        deferred = []
        for n in nodes:
            load, cnt, b = heapq.heappop(heap)
            newid[n] = k * RPC + b * 128 + cnt
            cnt += 1
            load += indeg[n]
            if cnt < 128:
                heapq.heappush(heap, (load, cnt, b))
            else:
                deferred.append(load)
        if heap:
            deferred.extend(h[0] for h in heap)
        max_block_load = max(max_block_load, max(deferred))
    T_B = int(np.ceil(max(max_block_load, 1) / 128.0))
    assert T_B * 128 >= max_block_load
    TPC = BPC * T_B  # tiles per core

    # --- real edges grouped by dst global block (self-loops handled
    # separately via the identity matmul on the core's own rows) ---
    nd = newid[dst]
    blk = nd // 128          # global block id
    order = np.argsort(blk, kind="stable")
    es, nd, blk = src[order], nd[order], blk[order]
    counts = np.bincount(blk, minlength=NB)
    assert counts.max() <= T_B * 128

    cap = T_B * 128
    src_l1 = np.zeros((NB, cap), dtype=np.int64)       # original src ids (pad 0)
    src_l23 = np.zeros((NB, cap), dtype=np.int64)      # table row ids (pad 0)
    dst_rel = np.full((NB, cap), -1.0, dtype=np.float32)  # pos in block (pad -1)
    valid = np.zeros((NB, cap), dtype=bool)
    starts = np.concatenate([[0], np.cumsum(counts)])
    for b in range(NB):
        s0, s1 = starts[b], starts[b + 1]
        c = s1 - s0
        src_l1[b, :c] = es[s0:s1]
        src_l23[b, :c] = newid[es[s0:s1]]
        dst_rel[b, :c] = (nd[s0:s1] % 128).astype(np.float32)
        valid[b, :c] = True

    # --- dinv per local row (0 for empty slots) ---
    dinv_rows = np.zeros(NPAD, dtype=np.float32)
    dinv_rows[newid] = dinv

    # --- pooling: per core, 8 graph slots x SLOTS_G blocks of 128 local rows ---
    SLOTS_G = int(np.ceil(gsizes.max() / 128.0))
    pool_rows = np.full((NC, 8, SLOTS_G * 128), RPC, dtype=np.int64)  # pad -> zero row
    for g in range(G):
        k, j = graph_core[g], graph_slot[g]
        rows = newid[np.where(batch == g)[0]] - k * RPC
        assert rows.min() >= 0 and rows.max() < RPC
        pool_rows[k, j, : len(rows)] = rows

    bf16 = ml_dtypes.bfloat16
    x_scaled = (tree_x * dinv[:, None]).astype(np.float32)
    iota_row = np.tile(np.arange(128, dtype=np.float32), (128, 1))
    ident = np.eye(128, dtype=np.float32).astype(bf16)

    # own rows of x' per core: xo[p, b, :] = x'[node at local row (b, p)]
    xo_all = np.zeros((NPAD, IN), dtype=np.float32)
    xo_all[newid] = x_scaled

    wkeys = ["W1", "b1", "W2", "b2", "W3", "b3", "Wf1", "bf1", "Wf2", "bf2",
             "Wf3", "bf3", "Wo", "bo"]
    weights = {k: np.asarray(inputs[k], dtype=np.float32) for k in wkeys}

    in_maps = []
    for k in range(NC):
        bsl = slice(k * BPC, (k + 1) * BPC)
        # host-pregathered edge-ordered x' (zeros in pad slots)
        xg = x_scaled[src_l1[bsl].reshape(-1)]
        xg[~valid[bsl].reshape(-1)] = 0.0
        xo = xo_all[k * RPC:(k + 1) * RPC].reshape(BPC, 128, IN)
        m = {
            "xg": xg.astype(bf16),
            "xo": np.ascontiguousarray(xo.transpose(1, 0, 2)).astype(bf16),
            "idx_l23": _wrap16(src_l23[bsl].reshape(-1)),
            "pool_idx": _wrap16(pool_rows[k].reshape(-1)),
            "dst_rel": dst_rel[bsl].reshape(-1, 128).T.copy(),  # [128, TPC]
            "dinv_own": dinv_rows[k * RPC:(k + 1) * RPC].reshape(BPC, 128).T.copy(),
            "iota_row": iota_row,
            "ident": ident,
            "W1": weights["W1"].astype(bf16),
            "W2": weights["W2"].astype(bf16),
            "W3": weights["W3"].astype(bf16),
            "b1": np.tile(weights["b1"][None, :], (128, 1)).astype(bf16),
            "b2": np.tile(weights["b2"][None, :], (128, 1)).astype(bf16),
            "b3": np.tile(weights["b3"][None, :], (128, 1)).astype(bf16),
            "Wf1": weights["Wf1"].astype(bf16),
            "Wf2": weights["Wf2"].astype(bf16),
            "Wf3": weights["Wf3"].astype(bf16),
            "bf1": weights["bf1"].reshape(4, 128).T.copy(),
            "bf2": weights["bf2"].reshape(4, 128).T.copy(),
            "bf3": weights["bf3"].reshape(4, 128).T.copy(),
            "Wo": weights["Wo"].astype(bf16),
            "bo": weights["bo"][:, None].copy(),
        }
        in_maps.append(m)

    meta = dict(T_B=T_B, SLOTS_G=SLOTS_G, groups=groups,
                newid=newid, dinv=dinv, graph_core=graph_core,
                graph_slot=graph_slot)
    return in_maps, meta


def assemble_output(core_outs, meta):
    """core_outs: list of 8 arrays [ACTD, 8] -> full [64, ACTD]."""
    out = np.zeros((G, ACTD), dtype=np.float32)
    for k in range(NC):
        for j, g in enumerate(meta["groups"][k]):
            out[g] = core_outs[k][:, j]
    return out


from contextlib import ExitStack
import concourse.bass as bass
import concourse.bacc as bacc
import concourse.mybir as mybir
import concourse.tile as tile

I32 = mybir.dt.int32
I16 = mybir.dt.int16
F32 = mybir.dt.float32
BF16 = mybir.dt.bfloat16
RELU = mybir.ActivationFunctionType.Relu
ADD = mybir.AluOpType.add
ISEQ = mybir.AluOpType.is_equal


def build(T_B, SLOTS_G):
    TPC = BPC * T_B
    GTILES = 8   # tiles per gather call: 1024 idx = SWDGE per-call cap

    nc = bacc.Bacc("TRN2", num_devices=NC)
    d = {}

    def param(name, shape, dt=F32):
        d[name] = nc.declare_dram_parameter(name, shape, dt, isOutput=False)

    param("xg", [TPC * 128, IN], BF16)
    param("xo", [128, BPC, IN], BF16)
    param("idx_l23", [128, TPC * 8], I16)
    param("pool_idx", [128, 8 * SLOTS_G * 8], I16)
    param("dst_rel", [128, TPC])
    param("dinv_own", [128, BPC])
    param("iota_row", [128, 128])
    param("ident", [128, 128], BF16)
    param("W1", [IN, HID], BF16)
    param("W2", [HID, HID], BF16)
    param("W3", [HID, HID], BF16)
    for b in ["b1", "b2", "b3"]:
        param(b, [128, HID], BF16)
    for w in ["Wf1", "Wf2", "Wf3"]:
        param(w, [HID, HID], BF16)
    for b in ["bf1", "bf2", "bf3"]:
        param(b, [128, 4])
    param("Wo", [HID, ACTD], BF16)
    param("bo", [ACTD, 1])
    out = nc.declare_dram_parameter("out", [ACTD, 8], F32, isOutput=True)

    with tile.TileContext(nc) as tc, ExitStack() as ctx:
        cpool = ctx.enter_context(tc.tile_pool(name="const", bufs=1))
        dram = ctx.enter_context(tc.tile_pool(name="dram", bufs=1, space="DRAM"))
        gpool = ctx.enter_context(tc.tile_pool(name="gather", bufs=3))
        opool = ctx.enter_context(tc.tile_pool(name="own", bufs=3))
        apool = ctx.enter_context(tc.tile_pool(name="agg", bufs=4))
        hpool = ctx.enter_context(tc.tile_pool(name="hrow", bufs=4))
        ppool_u = ctx.enter_context(tc.tile_pool(name="psum_u", bufs=3, space="PSUM"))
        ppool_t = ctx.enter_context(tc.tile_pool(name="psum_t", bufs=2, space="PSUM"))
        ppool_d = ctx.enter_context(tc.tile_pool(name="psum_d", bufs=2, space="PSUM"))

        # ---- DRAM intermediates ----
        h_own = dram.tile([RPC, HID], BF16, name="h_own")
        hf1 = dram.tile([NPAD, HID], BF16, addr_space="Shared", name="hf1")
        hf2 = dram.tile([NPAD, HID], BF16, addr_space="Shared", name="hf2")
        h3d = dram.tile([RPC + 128, HID], BF16, name="h3d")

        # ---- constants to SBUF ----
        def load(name, shape, dt=F32):
            t = cpool.tile(shape, dt, name=name)
            nc.sync.dma_start(out=t[:], in_=d[name][:])
            return t

        iota = load("iota_row", [128, 128])
        ident = load("ident", [128, 128], BF16)
        idx23 = load("idx_l23", [128, TPC * 8], I16)
        pidx = load("pool_idx", [128, 8 * SLOTS_G * 8], I16)
        drel = load("dst_rel", [128, TPC])
        dinv = load("dinv_own", [128, BPC])
        xo = load("xo", [128, BPC, IN], BF16)
        btiles = {l: load(f"b{l}", [128, HID], BF16) for l in (1, 2, 3)}
        bfs = {f: load(f"bf{f}", [128, 4]) for f in (1, 2, 3)}
        bo = load("bo", [ACTD, 1])

        W1sb = load("W1", [128, HID], BF16)
        Wsb = {}
        for l, wn in ((2, "W2"), (3, "W3")):
            t = cpool.tile([128, 4, HID], BF16, name=wn + "sb")
            for c in range(4):
                nc.sync.dma_start(out=t[:, c, :], in_=d[wn][c * 128:(c + 1) * 128, :])
            Wsb[l] = t
        Wfsb = {}
        for f in (1, 2, 3):
            t = cpool.tile([128, 4, HID], BF16, name=f"Wf{f}sb")
            for c in range(4):
                nc.sync.dma_start(out=t[:, c, :], in_=d[f"Wf{f}"][c * 128:(c + 1) * 128, :])
            Wfsb[f] = t
        Wosb = cpool.tile([128, 4, ACTD], BF16, name="Wosb")
        for c in range(4):
            nc.sync.dma_start(out=Wosb[:, c, :], in_=d["Wo"][c * 128:(c + 1) * 128, :])

        zt = cpool.tile([128, HID], BF16, name="zt")
        nc.vector.memset(zt[:], 0.0)
        nc.sync.dma_start(out=h3d[RPC:RPC + 128, :], in_=zt[:])

        # ---- prebuilt one-hot scatter matrices, shared by all three convs ----
        # Sp[:, gt, j] = 1.0 where the edge at (partition p, tile gt) targets
        # local dst position j within its block; pad entries (dst_rel=-1) are 0.
        Sp = cpool.tile([128, TPC, 128], BF16, name="Sp")
        iota_b = iota[:].rearrange("(p o) i -> p o i", o=1)
        for b in range(BPC):
            sl = slice(b * T_B, (b + 1) * T_B)
            nc.vector.tensor_tensor(
                out=Sp[:, sl, :],
                in0=drel[:, sl].rearrange("p (t o) -> p t o", o=1)
                    .to_broadcast([128, T_B, 128]),
                in1=iota_b.to_broadcast([128, T_B, 128]),
                op=ISEQ)

        # ---- one GCN conv layer ----
        def conv(l, src_dram, elem, Wt, btile, last):
            """src_dram: affine edge-ordered table (conv1) or row table to
            gather from (conv2/3). Self-loop term comes from own_rhs(b)."""
            nch = elem // 128
            gather = l > 1
            gbufs = {}

            def g_at(gt):
                gi = gt // GTILES
                if gi not in gbufs:
                    t0 = gi * GTILES
                    nt = min(GTILES, TPC - t0)
                    g = gpool.tile([128, nt, elem], BF16, name="g", tag="g")
                    if gather:
                        nc.gpsimd.dma_gather(
                            out_ap=g[:], in_ap=src_dram[:],
                            idxs_ap=idx23[:, t0 * 8:(t0 + nt) * 8],
                            num_idxs=nt * 128, num_idxs_reg=nt * 128,
                            elem_size=elem)
                    else:
                        nc.sync.dma_start(
                            out=g[:],
                            in_=src_dram[t0 * 128:(t0 + nt) * 128, :]
                                .rearrange("(c p) e -> p c e", p=128))
                    gbufs[gi] = g
                return gbufs[gi][:, gt % GTILES, :]

            for b in range(BPC):
                if l == 1:
                    own = xo[:, b, :]
                else:
                    ot = opool.tile([128, HID], BF16, name="ho", tag="ho")
                    nc.sync.dma_start(out=ot[:],
                                      in_=h_own[b * 128:(b + 1) * 128, :])
                    own = ot[:]
                u = ppool_u.tile([128, elem], F32, name="u", tag="u")
                # self-loop: u starts as I @ own (h'[d] = dinv_d * h[d])
                nc.tensor.matmul(u[:], lhsT=ident[:], rhs=own,
                                 start=True, stop=False)
                for t in range(T_B):
                    gt = b * T_B + t
                    nc.tensor.matmul(u[:], lhsT=Sp[:, gt, :], rhs=g_at(gt),
                                     start=False, stop=(t == T_B - 1))
                agg = apool.tile([128, elem], BF16, name="agg", tag="agg")
                nc.vector.tensor_scalar_mul(out=agg[:], in0=u[:],
                                            scalar1=dinv[:, b:b + 1])
                tp = ppool_t.tile([128, elem], BF16, name="tp", tag="tp")
                for c in range(nch):
                    nc.tensor.transpose(tp[:, c * 128:(c + 1) * 128],
                                        agg[:, c * 128:(c + 1) * 128], ident[:])
                aggT = apool.tile([128, elem], BF16, name="aggT", tag="aggT")
                nc.scalar.copy(out=aggT[:], in_=tp[:])
                hp = ppool_d.tile([128, HID], F32, name="hp", tag="hp")
                for c in range(nch):
                    Wc = Wt[:, c, :] if nch > 1 else Wt[:, :]
                    nc.tensor.matmul(hp[:], lhsT=aggT[:, c * 128:(c + 1) * 128],
                                     rhs=Wc, start=(c == 0), stop=(c == nch - 1))
                hb = hpool.tile([128, HID], F32, name="hb", tag="hb")
                nc.vector.tensor_tensor(out=hb[:], in0=hp[:], in1=btile[:],
                                        op=ADD)
                hr = hpool.tile([128, HID], BF16, name="hr", tag="hr")
                if last:
                    nc.scalar.activation(out=hr[:], in_=hb[:], func=RELU)
                    nc.sync.dma_start(out=h3d[b * 128:(b + 1) * 128, :],
                                      in_=hr[:])
                else:
                    # relu(h + b) * dinv == relu((h + b) * dinv), dinv >= 0
                    nc.scalar.activation(out=hr[:], in_=hb[:], func=RELU,
                                         scale=dinv[:, b:b + 1])
                    nc.sync.dma_start(out=h_own[b * 128:(b + 1) * 128, :],
                                      in_=hr[:])

        def allgather(hf):
            nc.gpsimd.collective_compute(
                "AllGather", mybir.AluOpType.bypass,
                replica_groups=[list(range(NC))],
                ins=[h_own[:]], outs=[hf[:]])

        with nc.named_scope("conv1"):
            conv(1, d["xg"], IN, W1sb, btiles[1], last=False)
        with nc.named_scope("ag1"):
            allgather(hf1)
        with nc.named_scope("conv2"):
            conv(2, hf1, HID, Wsb[2], btiles[2], last=False)
        with nc.named_scope("ag2"):
            allgather(hf2)
        with nc.named_scope("conv3"):
            conv(3, hf2, HID, Wsb[3], btiles[3], last=True)

        # ---- pooling: per graph slot, one transpose-mode gather (rows land
        # as columns) + one reduce_max over its columns ----
        nc.enter_named_scope("pool", False)
        ngr = SLOTS_G * 128   # gathered rows per graph slot (<= 1024)
        assert ngr <= 1024
        pooled = cpool.tile([128, 4, 8], BF16, name="pooled")
        for j in range(8):
            pg = gpool.tile([128, 4, ngr], BF16, name="pg", tag="pg")
            nc.gpsimd.dma_gather(
                out_ap=pg[:], in_ap=h3d[:],
                idxs_ap=pidx[:, j * ngr // 16:(j + 1) * ngr // 16],
                num_idxs=ngr, num_idxs_reg=ngr, elem_size=HID, transpose=True)
            nc.vector.reduce_max(out=pooled[:, :, j:j + 1], in_=pg[:],
                                 axis=mybir.AxisListType.X)

        # ---- MLP head (per-core on its 8 graphs) ----
        xcur = pooled[:].rearrange("p c j -> p (c j)")
        for f in (1, 2, 3):
            hp2 = ppool_d.tile([128, 32], F32, name="hp2", tag="hp")
            for co in range(4):
                for ci in range(4):
                    nc.tensor.matmul(
                        hp2[:, co * 8:(co + 1) * 8],
                        lhsT=Wfsb[f][:, ci, co * 128:(co + 1) * 128],
                        rhs=xcur[:, ci * 8:(ci + 1) * 8],
                        start=(ci == 0), stop=(ci == 3))
            xnext = cpool.tile([128, 32], BF16, name=f"x{f}")
            for co in range(4):
                nc.scalar.activation(out=xnext[:, co * 8:(co + 1) * 8],
                                     in_=hp2[:, co * 8:(co + 1) * 8], func=RELU,
                                     bias=bfs[f][:, co:co + 1])
            xcur = xnext[:]
        po = ppool_d.tile([ACTD, 8], F32, name="po", tag="hp")
        for ci in range(4):
            nc.tensor.matmul(po[:], lhsT=Wosb[:, ci, :],
                             rhs=xcur[:, ci * 8:(ci + 1) * 8],
                             start=(ci == 0), stop=(ci == 3))
        nc.leave_named_scope("pool", None, False)
        osb = cpool.tile([ACTD, 8], F32, name="osb")
        nc.vector.tensor_scalar_add(out=osb[:], in0=po[:], scalar1=bo[:, 0:1])
        nc.sync.dma_start(out=out[:], in_=osb[:])

    nc.compile()
    return nc


_CACHE = {}


def kernel(**inputs) -> np.ndarray:
    in_maps, meta = prep(inputs)
    key = (meta["T_B"], meta["SLOTS_G"])
    if key not in _CACHE:
        _CACHE[key] = build(meta["T_B"], meta["SLOTS_G"])
    nc = _CACHE[key]
    from concourse.bass_utils import run_bass_kernel_spmd
    res = run_bass_kernel_spmd(nc, in_maps, list(range(NC)))
    core_outs = [res.results[k]["out"] for k in range(NC)]
    return assemble_output(core_outs, meta)


# revision 26
# speedup vs baseline: 1.0230x; 1.0230x over previous
"""Distributed GCN (3x GCNConv + global_max_pool + MLP head) on 8 Trainium2
NeuronCores via concourse Bass/Tile SPMD. Graph-parallel: 8 graphs/core, node
rows block-packed per core balancing per-block in-edge counts.

Per layer, edge messages are segment-summed with one-hot PE matmuls (prebuilt
S matrices shared by all three convs). Conv1 reads a host-pregathered
edge-ordered x table with plain affine DMA (the gather permutation is known
at prep time). Conv2/3 gather bf16 h' rows with SWDGE dma_gather, one
1024-row call per block (the Q7 descriptor-generation cap). Self-loops are
excluded from the gather streams: their contribution is one identity matmul
per block from an affine read of the core's own rows. bf16 h' tables are
exchanged with an 8-rank AllGather between layers; per-graph max-pool via
transpose-mode dma_gather + reduce_max; per-core MLP head on device. Host
only shards/unshards.
"""
import sys
sys.path.insert(0, "/opt/trn_rl_repo")
import numpy as np

N = 20000
E = 160000
G = 64
IN = 128
HID = 512
ACTD = 32
NC = 8
RPC = 2560           # rows per core
BPC = 20             # blocks per core
NB = NC * BPC        # 160 global blocks
NPAD = NC * RPC      # 20480


def _wrap16(a):
    """Flat index list [L] -> SWDGE idx layout [128, L/16] int16.

    dma_gather reads index i from (partition i%16, col i//16); the 16-row
    pattern is replicated across all 8 Q7 core groups (128 partitions).
    """
    a = np.asarray(a)
    assert a.size % 16 == 0
    w = a.reshape(-1, 16).T.astype(np.int16)   # [16, L/16]
    return np.tile(w, (8, 1)).copy()           # [128, L/16]


def prep(inputs):
    import ml_dtypes
    src = np.asarray(inputs["edge_index"][0], dtype=np.int64)
    dst = np.asarray(inputs["edge_index"][1], dtype=np.int64)
    batch = np.asarray(inputs["batch"], dtype=np.int64)
    tree_x = np.asarray(inputs["tree_x"], dtype=np.float32)

    indeg = np.bincount(dst, minlength=N).astype(np.float64)  # real in-edges
    deg = indeg + 1.0                                         # + self loop
    dinv = (1.0 / np.sqrt(deg)).astype(np.float32)

    gsizes = np.bincount(batch, minlength=G)
    # graphs -> cores: 8 consecutive per core; LPT fallback if any group > RPC
    groups = [list(range(8 * k, 8 * k + 8)) for k in range(NC)]
    if max(int(gsizes[g].sum()) for g in groups) > RPC:
        order = np.argsort(-gsizes)
        loads = [0] * NC
        counts = [0] * NC
        groups = [[] for _ in range(NC)]
        for g in order:
            k = min(range(NC), key=lambda i: (loads[i] if counts[i] < 8 else 1 << 60))
            groups[k].append(int(g))
            loads[k] += int(gsizes[g])
            counts[k] += 1
        assert max(loads) <= RPC, f"graph groups do not fit: {loads}"

    graph_core = np.zeros(G, dtype=np.int64)
    graph_slot = np.zeros(G, dtype=np.int64)
    for k in range(NC):
        for j, g in enumerate(groups[k]):
            graph_core[g] = k
            graph_slot[g] = j

    # --- node -> (core, block, slot) balancing per-block real in-edge load ---
    import heapq
    newid = np.full(N, -1, dtype=np.int64)
    node_core = graph_core[batch]
    max_block_load = 0
    for k in range(NC):
        nodes = np.where(node_core == k)[0]
        nodes = nodes[np.argsort(-indeg[nodes], kind="stable")]
        heap = [(0.0, 0, b) for b in range(BPC)]  # (load, count, block)
        heapq.heapify(heap)
        deferred = []
        for n in nodes:
            load, cnt, b = heapq.heappop(heap)
            newid[n] = k * RPC + b * 128 + cnt
            cnt += 1
            load += indeg[n]
            if cnt < 128:
                heapq.heappush(heap, (load, cnt, b))
            else:
                deferred.append(load)
        if heap:
            deferred.extend(h[0] for h in heap)
        max_block_load = max(max_block_load, max(deferred))
    T_B = int(np.ceil(max(max_block_load, 1) / 128.0))
    assert T_B * 128 >= max_block_load
    TPC = BPC * T_B  # tiles per core

    # --- real edges grouped by dst global block (self-loops handled
    # separately via the identity matmul on the core's own rows) ---
    nd = newid[dst]
    blk = nd // 128          # global block id
    order = np.argsort(blk, kind="stable")
    es, nd, blk = src[order], nd[order], blk[order]
    counts = np.bincount(blk, minlength=NB)
    assert counts.max() <= T_B * 128

    cap = T_B * 128
    src_l1 = np.zeros((NB, cap), dtype=np.int64)       # original src ids (pad 0)
    src_l23 = np.zeros((NB, cap), dtype=np.int64)      # table row ids (pad 0)
    dst_rel = np.full((NB, cap), -1.0, dtype=np.float32)  # pos in block (pad -1)
    valid = np.zeros((NB, cap), dtype=bool)
    starts = np.concatenate([[0], np.cumsum(counts)])
    for b in range(NB):
        s0, s1 = starts[b], starts[b + 1]
        c = s1 - s0
        src_l1[b, :c] = es[s0:s1]
        src_l23[b, :c] = newid[es[s0:s1]]
        dst_rel[b, :c] = (nd[s0:s1] % 128).astype(np.float32)
        valid[b, :c] = True

    # --- dinv per local row (0 for empty slots) ---
    dinv_rows = np.zeros(NPAD, dtype=np.float32)
    dinv_rows[newid] = dinv

    # --- pooling: per core, 8 graph slots x SLOTS_G blocks of 128 local rows ---
    SLOTS_G = int(np.ceil(gsizes.max() / 128.0))
    pool_rows = np.full((NC, 8, SLOTS_G * 128), RPC, dtype=np.int64)  # pad -> zero row
    for g in range(G):
        k, j = graph_core[g], graph_slot[g]
        rows = newid[np.where(batch == g)[0]] - k * RPC
        assert rows.min() >= 0 and rows.max() < RPC
        pool_rows[k, j, : len(rows)] = rows

    bf16 = ml_dtypes.bfloat16
    x_scaled = (tree_x * dinv[:, None]).astype(np.float32)
    iota_row = np.tile(np.arange(128, dtype=np.float32), (128, 1))
    ident = np.eye(128, dtype=np.float32).astype(bf16)

    # own rows of x' per core: xo[p, b, :] = x'[node at local row (b, p)]
    xo_all = np.zeros((NPAD, IN), dtype=np.float32)
    xo_all[newid] = x_scaled

    wkeys = ["W1", "b1", "W2", "b2", "W3", "b3", "Wf1", "bf1", "Wf2", "bf2",
             "Wf3", "bf3", "Wo", "bo"]
    weights = {k: np.asarray(inputs[k], dtype=np.float32) for k in wkeys}

    in_maps = []
    for k in range(NC):
        bsl = slice(k * BPC, (k + 1) * BPC)
        # host-pregathered edge-ordered x' (zeros in pad slots)
        xg = x_scaled[src_l1[bsl].reshape(-1)]
        xg[~valid[bsl].reshape(-1)] = 0.0
        xo = xo_all[k * RPC:(k + 1) * RPC].reshape(BPC, 128, IN)
        m = {
            "xg": xg.astype(bf16),
            "xo": np.ascontiguousarray(xo.transpose(1, 0, 2)).astype(bf16),
            "idx_l23": _wrap16(src_l23[bsl].reshape(-1)),
            "pool_idx": _wrap16(pool_rows[k].reshape(-1)),
            "dst_rel": dst_rel[bsl].reshape(-1, 128).T.copy(),  # [128, TPC]
            "dinv_own": dinv_rows[k * RPC:(k + 1) * RPC].reshape(BPC, 128).T.copy(),
            "iota_row": iota_row,
            "ident": ident,
            "W1": weights["W1"].astype(bf16),
            "W2": weights["W2"].astype(bf16),
            "W3": weights["W3"].astype(bf16),
            "b1": np.tile(weights["b1"][None, :], (128, 1)).astype(bf16),
            "b2": np.tile(weights["b2"][None, :], (128, 1)).astype(bf16),
            "b3": np.tile(weights["b3"][None, :], (128, 1)).astype(bf16),
            "Wf1": weights["Wf1"].astype(bf16),
            "Wf2": weights["Wf2"].astype(bf16),
            "Wf3": weights["Wf3"].astype(bf16),
            "bf1": weights["bf1"].reshape(4, 128).T.copy(),
            "bf2": weights["bf2"].reshape(4, 128).T.copy(),
            "bf3": weights["bf3"].reshape(4, 128).T.copy(),
            "Wo": weights["Wo"].astype(bf16),
            "bo": weights["bo"][:, None].copy(),
        }
        in_maps.append(m)

    meta = dict(T_B=T_B, SLOTS_G=SLOTS_G, groups=groups,
                newid=newid, dinv=dinv, graph_core=graph_core,
                graph_slot=graph_slot)
    return in_maps, meta


def assemble_output(core_outs, meta):
    """core_outs: list of 8 arrays [ACTD, 8] -> full [64, ACTD]."""
    out = np.zeros((G, ACTD), dtype=np.float32)
    for k in range(NC):
        for j, g in enumerate(meta["groups"][k]):
            out[g] = core_outs[k][:, j]
    return out


from contextlib import ExitStack
import concourse.bass as bass
import concourse.bacc as bacc
import concourse.mybir as mybir
import concourse.tile as tile

I32 = mybir.dt.int32
I16 = mybir.dt.int16
F32 = mybir.dt.float32
BF16 = mybir.dt.bfloat16
RELU = mybir.ActivationFunctionType.Relu
ADD = mybir.AluOpType.add
ISEQ = mybir.AluOpType.is_equal


def build(T_B, SLOTS_G):
    TPC = BPC * T_B
    GTILES = 8   # tiles per gather call: 1024 idx = SWDGE per-call cap

    nc = bacc.Bacc("TRN2", num_devices=NC)
    d = {}

    def param(name, shape, dt=F32):
        d[name] = nc.declare_dram_parameter(name, shape, dt, isOutput=False)

    param("xg", [TPC * 128, IN], BF16)
    param("xo", [128, BPC, IN], BF16)
    param("idx_l23", [128, TPC * 8], I16)
    param("pool_idx", [128, 8 * SLOTS_G * 8], I16)
    param("dst_rel", [128, TPC])
    param("dinv_own", [128, BPC])
    param("iota_row", [128, 128])
    param("ident", [128, 128], BF16)
    param("W1", [IN, HID], BF16)
    param("W2", [HID, HID], BF16)
    param("W3", [HID, HID], BF16)
    for b in ["b1", "b2", "b3"]:
        param(b, [128, HID], BF16)
    for w in ["Wf1", "Wf2", "Wf3"]:
        param(w, [HID, HID], BF16)
    for b in ["bf1", "bf2", "bf3"]:
        param(b, [128, 4])
    param("Wo", [HID, ACTD], BF16)
    param("bo", [ACTD, 1])
    out = nc.declare_dram_parameter("out", [ACTD, 8], F32, isOutput=True)

    with tile.TileContext(nc) as tc, ExitStack() as ctx:
        cpool = ctx.enter_context(tc.tile_pool(name="const", bufs=1))
        dram = ctx.enter_context(tc.tile_pool(name="dram", bufs=1, space="DRAM"))
        gpool = ctx.enter_context(tc.tile_pool(name="gather", bufs=3))
        opool = ctx.enter_context(tc.tile_pool(name="own", bufs=2))
        apool = ctx.enter_context(tc.tile_pool(name="agg", bufs=3))
        hpool = ctx.enter_context(tc.tile_pool(name="hrow", bufs=3))
        ppool_u = ctx.enter_context(tc.tile_pool(name="psum_u", bufs=2, space="PSUM"))
        ppool_t = ctx.enter_context(tc.tile_pool(name="psum_t", bufs=2, space="PSUM"))
        ppool_d = ctx.enter_context(tc.tile_pool(name="psum_d", bufs=2, space="PSUM"))

        # ---- DRAM intermediates ----
        h_own = dram.tile([RPC, HID], BF16, name="h_own")
        hf1 = dram.tile([NPAD, HID], BF16, addr_space="Shared", name="hf1")
        hf2 = dram.tile([NPAD, HID], BF16, addr_space="Shared", name="hf2")
        h3d = dram.tile([RPC + 128, HID], BF16, name="h3d")

        # ---- constants to SBUF ----
        def load(name, shape, dt=F32):
            t = cpool.tile(shape, dt, name=name)
            nc.sync.dma_start(out=t[:], in_=d[name][:])
            return t

        iota = load("iota_row", [128, 128])
        ident = load("ident", [128, 128], BF16)
        idx23 = load("idx_l23", [128, TPC * 8], I16)
        pidx = load("pool_idx", [128, 8 * SLOTS_G * 8], I16)
        drel = load("dst_rel", [128, TPC])
        dinv = load("dinv_own", [128, BPC])
        xo = load("xo", [128, BPC, IN], BF16)
        btiles = {l: load(f"b{l}", [128, HID], BF16) for l in (1, 2, 3)}
        bfs = {f: load(f"bf{f}", [128, 4]) for f in (1, 2, 3)}
        bo = load("bo", [ACTD, 1])

        W1sb = load("W1", [128, HID], BF16)
        Wsb = {}
        for l, wn in ((2, "W2"), (3, "W3")):
            t = cpool.tile([128, 4, HID], BF16, name=wn + "sb")
            for c in range(4):
                nc.sync.dma_start(out=t[:, c, :], in_=d[wn][c * 128:(c + 1) * 128, :])
            Wsb[l] = t
        Wfsb = {}
        for f in (1, 2, 3):
            t = cpool.tile([128, 4, HID], BF16, name=f"Wf{f}sb")
            for c in range(4):
                nc.sync.dma_start(out=t[:, c, :], in_=d[f"Wf{f}"][c * 128:(c + 1) * 128, :])
            Wfsb[f] = t
        Wosb = cpool.tile([128, 4, ACTD], BF16, name="Wosb")
        for c in range(4):
            nc.sync.dma_start(out=Wosb[:, c, :], in_=d["Wo"][c * 128:(c + 1) * 128, :])

        zt = cpool.tile([128, HID], BF16, name="zt")
        nc.vector.memset(zt[:], 0.0)
        ones1 = cpool.tile([1, 128], BF16, name="ones1")
        nc.vector.memset(ones1[:], 1.0)
        nc.sync.dma_start(out=h3d[RPC:RPC + 128, :], in_=zt[:])

        # ---- prebuilt one-hot scatter matrices, shared by all three convs ----
        # Sp[:, gt, j] = 1.0 where the edge at (partition p, tile gt) targets
        # local dst position j within its block; pad entries (dst_rel=-1) are 0.
        Sp = cpool.tile([128, TPC, 128], BF16, name="Sp")
        iota_b = iota[:].rearrange("(p o) i -> p o i", o=1)
        for b in range(BPC):
            sl = slice(b * T_B, (b + 1) * T_B)
            nc.vector.tensor_tensor(
                out=Sp[:, sl, :],
                in0=drel[:, sl].rearrange("p (t o) -> p t o", o=1)
                    .to_broadcast([128, T_B, 128]),
                in1=iota_b.to_broadcast([128, T_B, 128]),
                op=ISEQ)

        # ---- one GCN conv layer ----
        def conv(l, src_dram, elem, Wt, btile, last):
            """src_dram: affine edge-ordered table (conv1) or row table to
            gather from (conv2/3). Self-loop term comes from own_rhs(b)."""
            nch = elem // 128
            gather = l > 1
            gbufs = {}

            def g_at(gt):
                gi = gt // GTILES
                if gi not in gbufs:
                    t0 = gi * GTILES
                    nt = min(GTILES, TPC - t0)
                    g = gpool.tile([128, nt, elem], BF16, name="g", tag="g")
                    if gather:
                        nc.gpsimd.dma_gather(
                            out_ap=g[:], in_ap=src_dram[:],
                            idxs_ap=idx23[:, t0 * 8:(t0 + nt) * 8],
                            num_idxs=nt * 128, num_idxs_reg=nt * 128,
                            elem_size=elem)
                    else:
                        nc.sync.dma_start(
                            out=g[:],
                            in_=src_dram[t0 * 128:(t0 + nt) * 128, :]
                                .rearrange("(c p) e -> p c e", p=128))
                    gbufs[gi] = g
                return gbufs[gi][:, gt % GTILES, :]

            for b in range(BPC):
                if l == 1:
                    own = xo[:, b, :]
                else:
                    ot = opool.tile([128, HID], BF16, name="ho", tag="ho")
                    nc.sync.dma_start(out=ot[:],
                                      in_=h_own[b * 128:(b + 1) * 128, :])
                    own = ot[:]
                u = ppool_u.tile([128, elem], F32, name="u", tag="u")
                # self-loop: u starts as I @ own (h'[d] = dinv_d * h[d])
                nc.tensor.matmul(u[:], lhsT=ident[:], rhs=own,
                                 start=True, stop=False)
                for t in range(T_B):
                    gt = b * T_B + t
                    nc.tensor.matmul(u[:], lhsT=Sp[:, gt, :], rhs=g_at(gt),
                                     start=False, stop=(t == T_B - 1))
                agg = apool.tile([128, elem], BF16, name="agg", tag="agg")
                nc.vector.tensor_scalar_mul(out=agg[:], in0=u[:],
                                            scalar1=dinv[:, b:b + 1])
                tp = ppool_t.tile([128, elem], BF16, name="tp", tag="tp")
                for c in range(nch):
                    nc.tensor.transpose(tp[:, c * 128:(c + 1) * 128],
                                        agg[:, c * 128:(c + 1) * 128], ident[:])
                aggT = apool.tile([128, elem], BF16, name="aggT", tag="aggT")
                nc.scalar.copy(out=aggT[:], in_=tp[:])
                hp = ppool_d.tile([128, HID], F32, name="hp", tag="hp")
                for c in range(nch):
                    Wc = Wt[:, c, :] if nch > 1 else Wt[:, :]
                    nc.tensor.matmul(hp[:], lhsT=aggT[:, c * 128:(c + 1) * 128],
                                     rhs=Wc, start=(c == 0), stop=False)
                # bias folded into the PSUM accumulation (adds b to every row)
                nc.tensor.matmul(hp[:], lhsT=ones1[:], rhs=btile[0:1, :],
                                 start=False, stop=True)
                hr = hpool.tile([128, HID], BF16, name="hr", tag="hr")
                if last:
                    nc.scalar.activation(out=hr[:], in_=hp[:], func=RELU)
                    nc.sync.dma_start(out=h3d[b * 128:(b + 1) * 128, :],
                                      in_=hr[:])
                else:
                    # relu(h + b) * dinv == relu((h + b) * dinv), dinv >= 0
                    nc.scalar.activation(out=hr[:], in_=hp[:], func=RELU,
                                         scale=dinv[:, b:b + 1])
                    nc.sync.dma_start(out=h_own[b * 128:(b + 1) * 128, :],
                                      in_=hr[:])

        def allgather(hf):
            nc.gpsimd.collective_compute(
                "AllGather", mybir.AluOpType.bypass,
                replica_groups=[list(range(NC))],
                ins=[h_own[:]], outs=[hf[:]])

        with nc.named_scope("conv1"):
            conv(1, d["xg"], IN, W1sb, btiles[1], last=False)
        with nc.named_scope("ag1"):
            allgather(hf1)
        with nc.named_scope("conv2"):
            conv(2, hf1, HID, Wsb[2], btiles[2], last=False)
        with nc.named_scope("ag2"):
            allgather(hf2)
        with nc.named_scope("conv3"):
            conv(3, hf2, HID, Wsb[3], btiles[3], last=True)

        # ---- pooling: per graph slot, one transpose-mode gather (rows land
        # as columns) + one reduce_max over its columns ----
        nc.enter_named_scope("pool", False)
        ngr = SLOTS_G * 128   # gathered rows per graph slot (<= 1024)
        assert ngr <= 1024
        pooled = cpool.tile([128, 4, 8], BF16, name="pooled")
        for j in range(8):
            pg = gpool.tile([128, 4, ngr], BF16, name="pg", tag="pg")
            nc.gpsimd.dma_gather(
                out_ap=pg[:], in_ap=h3d[:],
                idxs_ap=pidx[:, j * ngr // 16:(j + 1) * ngr // 16],
                num_idxs=ngr, num_idxs_reg=ngr, elem_size=HID, transpose=True)
            nc.vector.reduce_max(out=pooled[:, :, j:j + 1], in_=pg[:],
                                 axis=mybir.AxisListType.X)

        # ---- MLP head (per-core on its 8 graphs) ----
        xcur = pooled[:].rearrange("p c j -> p (c j)")
        for f in (1, 2, 3):
            hp2 = ppool_d.tile([128, 32], F32, name="hp2", tag="hp")
            for co in range(4):
                for ci in range(4):
                    nc.tensor.matmul(
                        hp2[:, co * 8:(co + 1) * 8],
                        lhsT=Wfsb[f][:, ci, co * 128:(co + 1) * 128],
                        rhs=xcur[:, ci * 8:(ci + 1) * 8],
                        start=(ci == 0), stop=(ci == 3))
            xnext = cpool.tile([128, 32], BF16, name=f"x{f}")
            for co in range(4):
                nc.scalar.activation(out=xnext[:, co * 8:(co + 1) * 8],
                                     in_=hp2[:, co * 8:(co + 1) * 8], func=RELU,
                                     bias=bfs[f][:, co:co + 1])
            xcur = xnext[:]
        po = ppool_d.tile([ACTD, 8], F32, name="po", tag="hp")
        for ci in range(4):
            nc.tensor.matmul(po[:], lhsT=Wosb[:, ci, :],
                             rhs=xcur[:, ci * 8:(ci + 1) * 8],
                             start=(ci == 0), stop=(ci == 3))
        nc.leave_named_scope("pool", None, False)
        osb = cpool.tile([ACTD, 8], F32, name="osb")
        nc.vector.tensor_scalar_add(out=osb[:], in0=po[:], scalar1=bo[:, 0:1])
        nc.sync.dma_start(out=out[:], in_=osb[:])

    nc.compile()
    return nc


_CACHE = {}


def kernel(**inputs) -> np.ndarray:
    in_maps, meta = prep(inputs)
    key = (meta["T_B"], meta["SLOTS_G"])
    if key not in _CACHE:
        _CACHE[key] = build(meta["T_B"], meta["SLOTS_G"])
    nc = _CACHE[key]
    from concourse.bass_utils import run_bass_kernel_spmd
    res = run_bass_kernel_spmd(nc, in_maps, list(range(NC)))
    core_outs = [res.results[k]["out"] for k in range(NC)]
    return assemble_output(core_outs, meta)
